# revision 2
# baseline (speedup 1.0000x reference)
"""Continuous positional bias kernel v2 for Trainium2 (8 NeuronCores).

Per core (256 queries of one batch), processed in query PAIRS:
    h1 = relu(gamma + beta_q)      DVE tensor_scalar, bf16 (2x_1P, ~570ns/q)
    p2 = w2^T h1                   PE, 4x N=512 matmuls -> [128,2048] PSUM
                                   (4 banks: qA cols 0-1023, qB 1024-2047)
    h2 = relu(p2 + b2)             ONE ACT activation FD=2048 (amortized init;
                                   ACT is the fastest PSUM reader ~0.9ns/col)
    p3 = w3^T h2                   PE, 4 col-tiled M=8 matmuls (concurrent,
                                   ~4ns start skew), output EMBEDDED in the
                                   pair's own p2 PSUM tile [:, :512] after
                                   the evacuation has read it (8-bank budget)
    stage = p3                     DVE fp32 copy -> SBUF, DMA out per round

Pipeline: iteration g issues h1/L2 for pair g+2 so DVE's stage(g) (ready
only after L3(g)) never blocks the next pair's h1 in the DVE FIFO, and PE's
L2(g+2) bank-0 matmul goes LAST (t order 1,2,3,0) since bank 0 of that
buffer is still being staged out.

Engine loads per pair (2 queries): ACT ~1.9us, DVE ~1.8us, PE ~1.6us.
GPSIMD deliberately unused: its tensor_scalar measures 18us/op and stalls
DVE via the shared SBUF port.
"""

import numpy as np

B, NQ, NK, H, HD = 2, 1024, 1024, 8, 128
NCORES = 8
CPB = NCORES // B          # cores per batch = 4
QPC = NQ // CPB            # queries per core = 256
KT = 512                   # k-tile (matmul moving free dim)
NPAIR = QPC // 2           # 128 pairs
RP = 8                     # pairs per staging round
NROUNDS = NPAIR // RP      # 16

_CACHE = {}


def _build_nc():
    from contextlib import ExitStack

    import concourse.bass as bass
    import concourse.tile as tile
    from concourse import bacc, mybir
    from concourse.alu_op_type import AluOpType

    f32 = mybir.dt.float32
    bf16 = mybir.dt.bfloat16
    Relu = mybir.ActivationFunctionType.Relu

    nc = bacc.Bacc(
        "TRN2",
        target_bir_lowering=False,
        debug=False,
        enable_asserts=True,
        num_devices=NCORES,
    )

    gamma_d = nc.dram_tensor("gamma", (HD, NK), bf16, kind="ExternalInput").ap()
    beta_d = nc.dram_tensor("beta", (HD, QPC), f32, kind="ExternalInput").ap()
    w2_d = nc.dram_tensor("w2", (HD, HD), bf16, kind="ExternalInput").ap()
    w3_d = nc.dram_tensor("w3", (HD, H), bf16, kind="ExternalInput").ap()
    b2_d = nc.dram_tensor("b2", (HD, 1), f32, kind="ExternalInput").ap()
    out_d = nc.dram_tensor("out", (H, QPC, NK), bf16, kind="ExternalOutput").ap()

    with tile.TileContext(nc) as tc:
        with ExitStack() as ctx:
            consts = ctx.enter_context(tc.tile_pool(name="consts", bufs=1))
            h1p = ctx.enter_context(tc.tile_pool(name="h1p", bufs=4))
            h2p = ctx.enter_context(tc.tile_pool(name="h2p", bufs=3))
            stagep = ctx.enter_context(tc.tile_pool(name="stagep", bufs=3))
            # ONE static [128,4096] f32 PSUM tile = all 8 banks, addressed by
            # slices. A 2-buf pool of [128,2048] tiles would add a
            # tile-granular WAR on recycle: L2(g+2) (all 4 matmuls) waits for
            # stage(g)'s bank-0 read. With range-based tracking on a single
            # tile, only the bank-0 matmul gets that edge.
            psp = ctx.enter_context(tc.tile_pool(name="psp", bufs=1, space="PSUM"))

            # b2 first (512B) so the ACT table preload can fire early
            b2 = consts.tile([HD, 1], f32)
            nc.sync.dma_start(b2, b2_d)
            gamma = consts.tile([HD, NK], bf16)
            nc.sync.dma_start(gamma, gamma_d)
            beta = consts.tile([HD, QPC], f32)
            nc.sync.dma_start(beta, beta_d)
            w2 = consts.tile([HD, HD], bf16)
            nc.sync.dma_start(w2, w2_d)
            w3 = consts.tile([HD, H], bf16)
            nc.sync.dma_start(w3, w3_d)
            # dummy activation: loads the Relu table set during input DMAs
            warm = consts.tile([HD, 1], f32)
            nc.scalar.activation(warm, b2, Relu)

            psum = psp.tile([128, 4096], f32, name="psum", tag="psum")

            def p2_of(g):
                """Pair g's 4-bank PSUM region (alternating halves)."""
                off = (g % 2) * 2 * NK
                return psum[:, off:off + 2 * NK]

            def make_h1(g):
                h1 = h1p.tile([HD, 2 * NK], bf16, name=f"h1_{g}", tag="h1")
                for jq in range(2):
                    q = 2 * g + jq
                    nc.vector.tensor_scalar(
                        h1[:, jq * NK:(jq + 1) * NK], gamma,
                        beta[:, q:q + 1], 0.0,
                        AluOpType.add, AluOpType.max)
                return h1

            def make_l2(g, h1):
                """L2 matmuls for pair g into its PSUM half.

                Bank-0 matmul (t=0) issues last: that bank holds pair
                g-2's L3 result until its stage copy completes.
                """
                p2 = p2_of(g)
                for t in (1, 2, 3, 0):
                    s = slice(t * KT, (t + 1) * KT)
                    nc.tensor.matmul(p2[:, s], w2, h1[:, s],
                                     start=True, stop=True)
                return p2

            # prologue: h1 two pairs ahead, L2 one pair ahead
            pend_h1 = {0: make_h1(0), 1: make_h1(1)}
            pend_p2 = {0: make_l2(0, pend_h1.pop(0))}

            for r in range(NROUNDS):
                stage = stagep.tile([128, RP * KT], bf16, name=f"st_{r}",
                                    tag="stage")
                for gg in range(RP):
                    g = r * RP + gg
                    if g + 2 < NPAIR:
                        pend_h1[g + 2] = make_h1(g + 2)
                    p2 = pend_p2.pop(g)
                    # evacuate pair g: one FD=2048 activation on ACT
                    h2 = h2p.tile([HD, 2 * NK], bf16, name=f"h2_{g}", tag="h2")
                    nc.scalar.activation(h2, p2, Relu, bias=b2)
                    # L2 of pair g+1 issues BEFORE L3(g) on the PE queue:
                    # L3(g) waits for the full evacuation, and a strict-FIFO
                    # PE would head-of-line-block L2(g+1) behind it,
                    # stalling evac(g+1).
                    if g + 1 < NPAIR:
                        pend_p2[g + 1] = make_l2(g + 1, pend_h1.pop(g + 1))
                    # L3: 4 col-tiled matmuls into this pair's own p2 tile
                    # (bank 0), legal once the evacuation has read it
                    for j in range(4):
                        nc.tensor.matmul(
                            p2[32 * j:32 * j + H, :KT],
                            w3,
                            h2[:, j * KT:(j + 1) * KT],
                            start=True, stop=True,
                            tile_position=(0, 32 * j),
                        )
                    # stage copy (fp32 PSUM -> bf16 SBUF, DVE)
                    nc.vector.tensor_copy(stage[:, gg * KT:(gg + 1) * KT],
                                          p2[:, :KT])
                # output DMAs for this round (32 queries)
                q0 = r * 2 * RP
                for j in range(4):
                    dest = bass.AP(
                        tensor=out_d.tensor,
                        offset=out_d.offset
                        + (q0 + (j // 2)) * NK
                        + (j % 2) * KT,
                        ap=[[QPC * NK, H], [2 * NK, RP], [1, KT]],
                    )
                    nc.sync.dma_start(dest, stage[32 * j:32 * j + H, :])

    nc.compile()
    return nc


def _get_nc():
    if "nc" not in _CACHE:
        _CACHE["nc"] = _build_nc()
    return _CACHE["nc"]


def _bf16(x):
    try:
        import ml_dtypes
        return x.astype(ml_dtypes.bfloat16)
    except ImportError:
        import jax.numpy as jnp
        return np.asarray(jnp.asarray(x, dtype=jnp.bfloat16))


def make_in_maps(query_coords, key_coords, w1, b1, w2, b2, w3):
    """Host-side shard prep: per-core gamma/beta + replicated weights."""
    qc = np.asarray(query_coords, np.float32)
    kc = np.asarray(key_coords, np.float32)
    w1 = np.asarray(w1, np.float32)
    b1 = np.asarray(b1, np.float32)
    w2 = np.asarray(w2, np.float32)
    b2 = np.asarray(b2, np.float32)
    w3 = np.asarray(w3, np.float32)

    w2b = _bf16(np.ascontiguousarray(w2))
    w3b = _bf16(np.ascontiguousarray(w3))
    b2c = np.ascontiguousarray(b2.reshape(HD, 1))

    in_maps = []
    for c in range(NCORES):
        b = c // CPB
        q0 = (c % CPB) * QPC
        gamma = np.ascontiguousarray(-(kc[b] @ w1).T)            # (128, NK)
        beta = np.ascontiguousarray(
            (qc[b, q0:q0 + QPC] @ w1).T + b1[:, None]            # (128, QPC)
        )
        in_maps.append(
            {"gamma": _bf16(gamma), "beta": beta, "w2": w2b, "w3": w3b,
             "b2": b2c}
        )
    return in_maps


def assemble_output(results, b3):
    """Gather per-core [H, QPC, NK] results into (B, H, NQ, NK)."""
    b3 = np.asarray(b3, np.float32)
    out = np.empty((B, H, NQ, NK), np.float32)
    for c in range(NCORES):
        b = c // CPB
        q0 = (c % CPB) * QPC
        out[b, :, q0:q0 + QPC, :] = results[c]["out"].astype(np.float32)
    if np.any(b3):
        out += b3.reshape(1, H, 1, 1)
    return out


def kernel(**inputs):
    from concourse.bass_utils import run_bass_kernel_spmd

    in_maps = make_in_maps(
        inputs["query_coords"],
        inputs["key_coords"],
        inputs["w1"],
        inputs["b1"],
        inputs["w2"],
        inputs["b2"],
        inputs["w3"],
    )
    nc = _get_nc()
    res = run_bass_kernel_spmd(nc, in_maps, list(range(NCORES)))
    return assemble_output(res.results, inputs["b3"])


# revision 3
# speedup vs baseline: 1.0491x; 1.0491x over previous
"""Continuous positional bias kernel v2 for Trainium2 (8 NeuronCores).

Per core (256 queries of one batch), processed in query PAIRS:
    h1 = relu(gamma + beta_q)      DVE tensor_scalar, bf16 (2x_1P, ~570ns/q)
    p2 = w2^T h1                   PE, 4x N=512 matmuls -> [128,2048] PSUM
                                   (4 banks: qA cols 0-1023, qB 1024-2047)
    h2 = relu(p2 + b2)             ONE ACT activation FD=2048 (amortized init;
                                   ACT is the fastest PSUM reader ~0.9ns/col)
    p3 = w3^T h2                   PE, 4 col-tiled M=8 matmuls (concurrent,
                                   ~4ns start skew), output EMBEDDED in the
                                   pair's own p2 PSUM tile [:, :512] after
                                   the evacuation has read it (8-bank budget)
    stage = p3                     DVE cast fp32->bf16 -> SBUF, bf16 DMA out

Pipeline (steady state ~1950ns/pair, ACT-bound): h1 issues two pairs
ahead and L2 one pair ahead (right after the evacuation) so neither is
head-of-line blocked in its engine FIFO behind work that waits on the
current evacuation. L2's bank-0 matmul goes last (t order 1,2,3,0)
since that bank holds pair g-2's L3 result until its stage copy reads it.

Engine loads per pair (2 queries): ACT ~1.9us, DVE ~1.8us, PE ~1.6us.
GPSIMD deliberately unused: its tensor_scalar measures 18us/op and stalls
DVE via the shared SBUF port.
"""

import numpy as np

B, NQ, NK, H, HD = 2, 1024, 1024, 8, 128
NCORES = 8
CPB = NCORES // B          # cores per batch = 4
QPC = NQ // CPB            # queries per core = 256
KT = 512                   # k-tile (matmul moving free dim)
NPAIR = QPC // 2           # 128 pairs
RP = 8                     # pairs per staging round
NROUNDS = NPAIR // RP      # 16

_CACHE = {}


def _build_nc():
    from contextlib import ExitStack

    import concourse.bass as bass
    import concourse.tile as tile
    from concourse import bacc, mybir
    from concourse.alu_op_type import AluOpType

    f32 = mybir.dt.float32
    bf16 = mybir.dt.bfloat16
    Relu = mybir.ActivationFunctionType.Relu

    nc = bacc.Bacc(
        "TRN2",
        target_bir_lowering=False,
        debug=False,
        enable_asserts=True,
        num_devices=NCORES,
    )

    gamma_d = nc.dram_tensor("gamma", (HD, NK), bf16, kind="ExternalInput").ap()
    beta_d = nc.dram_tensor("beta", (HD, QPC), f32, kind="ExternalInput").ap()
    w2_d = nc.dram_tensor("w2", (HD, HD), bf16, kind="ExternalInput").ap()
    w3_d = nc.dram_tensor("w3", (HD, H), bf16, kind="ExternalInput").ap()
    b2_d = nc.dram_tensor("b2", (HD, 1), f32, kind="ExternalInput").ap()
    out_d = nc.dram_tensor("out", (H, QPC, NK), bf16, kind="ExternalOutput").ap()

    with tile.TileContext(nc) as tc:
        with ExitStack() as ctx:
            consts = ctx.enter_context(tc.tile_pool(name="consts", bufs=1))
            h1p = ctx.enter_context(tc.tile_pool(name="h1p", bufs=4))
            h2p = ctx.enter_context(tc.tile_pool(name="h2p", bufs=3))
            stagep = ctx.enter_context(tc.tile_pool(name="stagep", bufs=3))
            # ONE static [128,4096] f32 PSUM tile = all 8 banks, addressed by
            # slices. A 2-buf pool of [128,2048] tiles would add a
            # tile-granular WAR on recycle: L2(g+2) (all 4 matmuls) waits for
            # stage(g)'s bank-0 read. With range-based tracking on a single
            # tile, only the bank-0 matmul gets that edge.
            psp = ctx.enter_context(tc.tile_pool(name="psp", bufs=1, space="PSUM"))

            # b2 first (512B) so the ACT table preload can fire early
            b2 = consts.tile([HD, 1], f32)
            nc.sync.dma_start(b2, b2_d)
            gamma = consts.tile([HD, NK], bf16)
            nc.sync.dma_start(gamma, gamma_d)
            beta = consts.tile([HD, QPC], f32)
            nc.sync.dma_start(beta, beta_d)
            w2 = consts.tile([HD, HD], bf16)
            nc.sync.dma_start(w2, w2_d)
            w3 = consts.tile([HD, H], bf16)
            nc.sync.dma_start(w3, w3_d)
            # dummy activation: loads the Relu table set during input DMAs
            warm = consts.tile([HD, 1], f32)
            nc.scalar.activation(warm, b2, Relu)

            psum = psp.tile([128, 4096], f32, name="psum", tag="psum")

            def p2_of(g):
                """Pair g's 4-bank PSUM region (alternating halves)."""
                off = (g % 2) * 2 * NK
                return psum[:, off:off + 2 * NK]

            def make_h1(g):
                h1 = h1p.tile([HD, 2 * NK], bf16, name=f"h1_{g}", tag="h1")
                for jq in range(2):
                    q = 2 * g + jq
                    nc.vector.tensor_scalar(
                        h1[:, jq * NK:(jq + 1) * NK], gamma,
                        beta[:, q:q + 1], 0.0,
                        AluOpType.add, AluOpType.max)
                return h1

            def make_l2(g, h1):
                """L2 matmuls for pair g into its PSUM half.

                Bank-0 matmul (t=0) issues last: that bank holds pair
                g-2's L3 result until its stage copy completes.
                """
                p2 = p2_of(g)
                for t in (1, 2, 3, 0):
                    s = slice(t * KT, (t + 1) * KT)
                    nc.tensor.matmul(p2[:, s], w2, h1[:, s],
                                     start=True, stop=True)
                return p2

            # prologue: h1 two pairs ahead, L2 one pair ahead
            pend_h1 = {0: make_h1(0), 1: make_h1(1)}
            pend_p2 = {0: make_l2(0, pend_h1.pop(0))}

            for r in range(NROUNDS):
                stage = stagep.tile([128, RP * KT], bf16, name=f"st_{r}",
                                    tag="stage")
                for gg in range(RP):
                    g = r * RP + gg
                    if g + 2 < NPAIR:
                        pend_h1[g + 2] = make_h1(g + 2)
                    p2 = pend_p2.pop(g)
                    # evacuate pair g: one FD=2048 activation on ACT
                    h2 = h2p.tile([HD, 2 * NK], bf16, name=f"h2_{g}", tag="h2")
                    nc.scalar.activation(h2, p2, Relu, bias=b2)
                    # L2 of pair g+1 issues BEFORE L3(g) on the PE queue:
                    # L3(g) waits for the full evacuation, and a strict-FIFO
                    # PE would head-of-line-block L2(g+1) behind it,
                    # stalling evac(g+1).
                    if g + 1 < NPAIR:
                        pend_p2[g + 1] = make_l2(g + 1, pend_h1.pop(g + 1))
                    # L3: 4 col-tiled matmuls into this pair's own p2 tile
                    # (bank 0), legal once the evacuation has read it
                    for j in range(4):
                        nc.tensor.matmul(
                            p2[32 * j:32 * j + H, :KT],
                            w3,
                            h2[:, j * KT:(j + 1) * KT],
                            start=True, stop=True,
                            tile_position=(0, 32 * j),
                        )
                    # stage copy (fp32 PSUM -> bf16 SBUF, DVE)
                    nc.vector.tensor_copy(stage[:, gg * KT:(gg + 1) * KT],
                                          p2[:, :KT])
                # output DMAs for this round (32 queries)
                q0 = r * 2 * RP
                for j in range(4):
                    dest = bass.AP(
                        tensor=out_d.tensor,
                        offset=out_d.offset
                        + (q0 + (j // 2)) * NK
                        + (j % 2) * KT,
                        ap=[[QPC * NK, H], [2 * NK, RP], [1, KT]],
                    )
                    nc.sync.dma_start(dest, stage[32 * j:32 * j + H, :])

    nc.compile()
    return nc


def _get_nc():
    if "nc" not in _CACHE:
        _CACHE["nc"] = _build_nc()
    return _CACHE["nc"]


def _bf16(x):
    try:
        import ml_dtypes
        return x.astype(ml_dtypes.bfloat16)
    except ImportError:
        import jax.numpy as jnp
        return np.asarray(jnp.asarray(x, dtype=jnp.bfloat16))


def make_in_maps(query_coords, key_coords, w1, b1, w2, b2, w3):
    """Host-side shard prep: per-core gamma/beta + replicated weights."""
    qc = np.asarray(query_coords, np.float32)
    kc = np.asarray(key_coords, np.float32)
    w1 = np.asarray(w1, np.float32)
    b1 = np.asarray(b1, np.float32)
    w2 = np.asarray(w2, np.float32)
    b2 = np.asarray(b2, np.float32)
    w3 = np.asarray(w3, np.float32)

    w2b = _bf16(np.ascontiguousarray(w2))
    w3b = _bf16(np.ascontiguousarray(w3))
    b2c = np.ascontiguousarray(b2.reshape(HD, 1))

    in_maps = []
    for c in range(NCORES):
        b = c // CPB
        q0 = (c % CPB) * QPC
        gamma = np.ascontiguousarray(-(kc[b] @ w1).T)            # (128, NK)
        beta = np.ascontiguousarray(
            (qc[b, q0:q0 + QPC] @ w1).T + b1[:, None]            # (128, QPC)
        )
        in_maps.append(
            {"gamma": _bf16(gamma), "beta": beta, "w2": w2b, "w3": w3b,
             "b2": b2c}
        )
    return in_maps


def assemble_output(results, b3):
    """Gather per-core [H, QPC, NK] results into (B, H, NQ, NK)."""
    b3 = np.asarray(b3, np.float32)
    out = np.empty((B, H, NQ, NK), np.float32)
    for c in range(NCORES):
        b = c // CPB
        q0 = (c % CPB) * QPC
        out[b, :, q0:q0 + QPC, :] = results[c]["out"].astype(np.float32)
    if np.any(b3):
        out += b3.reshape(1, H, 1, 1)
    return out


def kernel(**inputs):
    from concourse.bass_utils import run_bass_kernel_spmd

    in_maps = make_in_maps(
        inputs["query_coords"],
        inputs["key_coords"],
        inputs["w1"],
        inputs["b1"],
        inputs["w2"],
        inputs["b2"],
        inputs["w3"],
    )
    nc = _get_nc()
    res = run_bass_kernel_spmd(nc, in_maps, list(range(NCORES)))
    return assemble_output(res.results, inputs["b3"])


# revision 5
# speedup vs baseline: 1.0600x; 1.0104x over previous
"""Continuous positional bias kernel v2 for Trainium2 (8 NeuronCores).

Per core (256 queries of one batch), processed in query PAIRS:
    h1 = relu(gamma + beta_q)      DVE tensor_scalar, bf16 (2x_1P, ~570ns/q)
    p2 = w2^T h1                   PE, 4x N=512 matmuls -> [128,2048] PSUM
                                   (4 banks: qA cols 0-1023, qB 1024-2047)
    h2 = relu(p2 + b2)             ONE ACT activation FD=2048 (amortized init;
                                   ACT is the fastest PSUM reader ~0.9ns/col)
    p3 = w3^T h2                   PE, 4 col-tiled M=8 matmuls (concurrent,
                                   ~4ns start skew), output EMBEDDED in the
                                   pair's own p2 PSUM tile [:, :512] after
                                   the evacuation has read it (8-bank budget)
    stage = p3                     DVE cast fp32->bf16 -> SBUF, bf16 DMA out

Pipeline (steady state ~1950ns/pair, ACT-bound): h1 issues two pairs
ahead and L2 one pair ahead (right after the evacuation) so neither is
head-of-line blocked in its engine FIFO behind work that waits on the
current evacuation. L2's bank-0 matmul goes last (t order 1,2,3,0)
since that bank holds pair g-2's L3 result until its stage copy reads it.

Engine loads per pair (2 queries): ACT ~1.9us, DVE ~1.8us, PE ~1.6us.
GPSIMD deliberately unused: its tensor_scalar measures 18us/op and stalls
DVE via the shared SBUF port.
"""

import numpy as np

B, NQ, NK, H, HD = 2, 1024, 1024, 8, 128
NCORES = 8
CPB = NCORES // B          # cores per batch = 4
QPC = NQ // CPB            # queries per core = 256
KT = 512                   # k-tile (matmul moving free dim)
NPAIR = QPC // 2           # 128 pairs
RP = 4                     # pairs per staging round (small rounds ->
                           # the post-last-evac DMA drain stays short)
NROUNDS = NPAIR // RP      # 32

_CACHE = {}


def _build_nc():
    from contextlib import ExitStack

    import concourse.bass as bass
    import concourse.tile as tile
    from concourse import bacc, mybir
    from concourse.alu_op_type import AluOpType

    f32 = mybir.dt.float32
    bf16 = mybir.dt.bfloat16
    Relu = mybir.ActivationFunctionType.Relu

    nc = bacc.Bacc(
        "TRN2",
        target_bir_lowering=False,
        debug=False,
        enable_asserts=True,
        num_devices=NCORES,
    )

    gamma_d = nc.dram_tensor("gamma", (HD, NK), bf16, kind="ExternalInput").ap()
    beta_d = nc.dram_tensor("beta", (HD, QPC), f32, kind="ExternalInput").ap()
    w2_d = nc.dram_tensor("w2", (HD, HD), bf16, kind="ExternalInput").ap()
    w3_d = nc.dram_tensor("w3", (HD, H), bf16, kind="ExternalInput").ap()
    b2_d = nc.dram_tensor("b2", (HD, 1), f32, kind="ExternalInput").ap()
    out_d = nc.dram_tensor("out", (H, QPC, NK), bf16, kind="ExternalOutput").ap()

    with tile.TileContext(nc) as tc:
        with ExitStack() as ctx:
            consts = ctx.enter_context(tc.tile_pool(name="consts", bufs=1))
            h1p = ctx.enter_context(tc.tile_pool(name="h1p", bufs=4))
            h2p = ctx.enter_context(tc.tile_pool(name="h2p", bufs=3))
            stagep = ctx.enter_context(tc.tile_pool(name="stagep", bufs=3))
            # ONE static [128,4096] f32 PSUM tile = all 8 banks, addressed by
            # slices. A 2-buf pool of [128,2048] tiles would add a
            # tile-granular WAR on recycle: L2(g+2) (all 4 matmuls) waits for
            # stage(g)'s bank-0 read. With range-based tracking on a single
            # tile, only the bank-0 matmul gets that edge.
            psp = ctx.enter_context(tc.tile_pool(name="psp", bufs=1, space="PSUM"))

            # gamma first (largest input, gates h1), then beta/b2/w2/w3
            gamma = consts.tile([HD, NK], bf16)
            nc.sync.dma_start(gamma, gamma_d)
            beta = consts.tile([HD, QPC], f32)
            nc.sync.dma_start(beta, beta_d)
            w2 = consts.tile([HD, HD], bf16)
            nc.sync.dma_start(w2, w2_d)
            b2 = consts.tile([HD, 1], f32)
            nc.sync.dma_start(b2, b2_d)
            w3 = consts.tile([HD, H], bf16)
            nc.sync.dma_start(w3, w3_d)
            # dummy activation: loads the Relu table set during input DMAs
            warm = consts.tile([HD, 1], f32)
            nc.scalar.activation(warm, b2, Relu)

            psum = psp.tile([128, 4096], f32, name="psum", tag="psum")

            # HAM pre-warm: ~48 tiny matmuls on a memset tile keep the PE
            # busy during the input-DMA window so the first real L2 runs at
            # 2.4GHz instead of the cold 1.2GHz gate. Output lands in bank 0
            # scratch that L2(0).t0 overwrites (start=True).
            dummy = consts.tile([128, 64], bf16)
            nc.vector.memset(dummy, 0.0)
            for _ in range(48):
                nc.tensor.matmul(psum[:64, :64], dummy[:, :64], dummy,
                                 start=True, stop=True)

            def p2_of(g):
                """Pair g's 4-bank PSUM region (alternating halves)."""
                off = (g % 2) * 2 * NK
                return psum[:, off:off + 2 * NK]

            def make_h1(g):
                h1 = h1p.tile([HD, 2 * NK], bf16, name=f"h1_{g}", tag="h1")
                for jq in range(2):
                    q = 2 * g + jq
                    nc.vector.tensor_scalar(
                        h1[:, jq * NK:(jq + 1) * NK], gamma,
                        beta[:, q:q + 1], 0.0,
                        AluOpType.add, AluOpType.max)
                return h1

            def make_l2(g, h1):
                """L2 matmuls for pair g into its PSUM half.

                Bank-0 matmul (t=0) issues last: that bank holds pair
                g-2's L3 result until its stage copy completes.
                """
                p2 = p2_of(g)
                for t in (1, 2, 3, 0):
                    s = slice(t * KT, (t + 1) * KT)
                    nc.tensor.matmul(p2[:, s], w2, h1[:, s],
                                     start=True, stop=True)
                return p2

            # prologue: h1 two pairs ahead, L2 one pair ahead
            pend_h1 = {0: make_h1(0), 1: make_h1(1)}
            pend_p2 = {0: make_l2(0, pend_h1.pop(0))}

            for r in range(NROUNDS):
                stage = stagep.tile([128, RP * KT], bf16, name=f"st_{r}",
                                    tag="stage")
                for gg in range(RP):
                    g = r * RP + gg
                    if g + 2 < NPAIR:
                        pend_h1[g + 2] = make_h1(g + 2)
                    p2 = pend_p2.pop(g)
                    # evacuate pair g: one FD=2048 activation on ACT
                    h2 = h2p.tile([HD, 2 * NK], bf16, name=f"h2_{g}", tag="h2")
                    nc.scalar.activation(h2, p2, Relu, bias=b2)
                    # L2 of pair g+1 issues BEFORE L3(g) on the PE queue:
                    # L3(g) waits for the full evacuation, and a strict-FIFO
                    # PE would head-of-line-block L2(g+1) behind it,
                    # stalling evac(g+1).
                    if g + 1 < NPAIR:
                        pend_p2[g + 1] = make_l2(g + 1, pend_h1.pop(g + 1))
                    # L3: 4 col-tiled matmuls into this pair's own p2 tile
                    # (bank 0), legal once the evacuation has read it
                    for j in range(4):
                        nc.tensor.matmul(
                            p2[32 * j:32 * j + H, :KT],
                            w3,
                            h2[:, j * KT:(j + 1) * KT],
                            start=True, stop=True,
                            tile_position=(0, 32 * j),
                        )
                    # stage copy (fp32 PSUM -> bf16 SBUF, DVE)
                    nc.vector.tensor_copy(stage[:, gg * KT:(gg + 1) * KT],
                                          p2[:, :KT])
                # output DMAs for this round (32 queries)
                q0 = r * 2 * RP
                for j in range(4):
                    dest = bass.AP(
                        tensor=out_d.tensor,
                        offset=out_d.offset
                        + (q0 + (j // 2)) * NK
                        + (j % 2) * KT,
                        ap=[[QPC * NK, H], [2 * NK, RP], [1, KT]],
                    )
                    nc.sync.dma_start(dest, stage[32 * j:32 * j + H, :])

    nc.compile()
    return nc


def _get_nc():
    if "nc" not in _CACHE:
        _CACHE["nc"] = _build_nc()
    return _CACHE["nc"]


def _bf16(x):
    try:
        import ml_dtypes
        return x.astype(ml_dtypes.bfloat16)
    except ImportError:
        import jax.numpy as jnp
        return np.asarray(jnp.asarray(x, dtype=jnp.bfloat16))


def make_in_maps(query_coords, key_coords, w1, b1, w2, b2, w3):
    """Host-side shard prep: per-core gamma/beta + replicated weights."""
    qc = np.asarray(query_coords, np.float32)
    kc = np.asarray(key_coords, np.float32)
    w1 = np.asarray(w1, np.float32)
    b1 = np.asarray(b1, np.float32)
    w2 = np.asarray(w2, np.float32)
    b2 = np.asarray(b2, np.float32)
    w3 = np.asarray(w3, np.float32)

    w2b = _bf16(np.ascontiguousarray(w2))
    w3b = _bf16(np.ascontiguousarray(w3))
    b2c = np.ascontiguousarray(b2.reshape(HD, 1))

    in_maps = []
    for c in range(NCORES):
        b = c // CPB
        q0 = (c % CPB) * QPC
        gamma = np.ascontiguousarray(-(kc[b] @ w1).T)            # (128, NK)
        beta = np.ascontiguousarray(
            (qc[b, q0:q0 + QPC] @ w1).T + b1[:, None]            # (128, QPC)
        )
        in_maps.append(
            {"gamma": _bf16(gamma), "beta": beta, "w2": w2b, "w3": w3b,
             "b2": b2c}
        )
    return in_maps


def assemble_output(results, b3):
    """Gather per-core [H, QPC, NK] results into (B, H, NQ, NK)."""
    b3 = np.asarray(b3, np.float32)
    out = np.empty((B, H, NQ, NK), np.float32)
    for c in range(NCORES):
        b = c // CPB
        q0 = (c % CPB) * QPC
        out[b, :, q0:q0 + QPC, :] = results[c]["out"].astype(np.float32)
    if np.any(b3):
        out += b3.reshape(1, H, 1, 1)
    return out


def kernel(**inputs):
    from concourse.bass_utils import run_bass_kernel_spmd

    in_maps = make_in_maps(
        inputs["query_coords"],
        inputs["key_coords"],
        inputs["w1"],
        inputs["b1"],
        inputs["w2"],
        inputs["b2"],
        inputs["w3"],
    )
    nc = _get_nc()
    res = run_bass_kernel_spmd(nc, in_maps, list(range(NCORES)))
    return assemble_output(res.results, inputs["b3"])
